# revision 13
# baseline (speedup 1.0000x reference)
"""ExternalAttention Trainium2 Bass kernel.

Math (per batch b, with N = H*W = 4096 tokens, C = 512, K = 64):
    x      = inputs @ w1 + b1          [N, C]
    logits = x @ m0                    [N, K]
    attn   = softmax(logits, axis=N)   (then L1-normalize over N — a no-op
                                        divide by 1 + 1e-9, skipped)
    y      = attn @ m1 @ w2            [N, C]
    out    = relu(BN_affine(y) + inputs)

Host-side folds (all tiny C x C / C x K matrices):
    wm    = w1 @ m0                                 [C, K]   (b1 @ m0 shifts each
            softmax column by a constant -> softmax-invariant, dropped)
    scale = gamma / sqrt(bn_var + eps); shift = beta - bn_mean * scale
    w2m   = m1 @ (w2 * scale)                       [K, C]
    => out = relu(colsoftmax(inputs @ wm) @ w2m + shift + inputs)

Device kernel (per core, 2 batches, data-parallel over B=16 on 8 cores):
    - load A = inputs[b] in [128, 8, 512] tiles (2MB DMAs)
    - PE-transpose A tiles into A^T scratch (f32r), mm1: logitsT[64, N] = wm^T A^T
    - column softmax along the free axis (DVE reduce-max, ACT exp with
      accumulated sum, DVE reciprocal + scale)
    - mm2 per 128-token tile: psum = attn_aug^T @ w2m_aug (+shift via ones row)
      then += identity @ A_tile (residual add on the PE), ACT relu -> SBUF
    - store (2MB DMAs)
"""

import os
import sys
from contextlib import ExitStack

import numpy as np

for _p in ("/opt/trn_rl_repo", os.path.expanduser("~/.axon_site/_ro/trn_rl_repo")):
    if os.path.isdir(_p) and _p not in sys.path:
        sys.path.insert(0, _p)

import concourse.bass as bass
import concourse.mybir as mybir
import concourse.tile as tile
from concourse import bacc
from concourse.bass import ts
from concourse.bass_utils import run_bass_kernel_spmd
from concourse.masks import make_identity

B, H, W, C, K = 16, 64, 64, 512, 64
N = H * W  # 4096 tokens
BN_EPS = 1e-3
NCORES = 8
BPC = B // NCORES  # batches per core = 2

F32 = mybir.dt.float32
F32R = mybir.dt.float32r

NT = N // 128        # 32 token tiles per batch
NG = 4               # token groups of 1024 (8 tiles) per batch
TPG = NT // NG       # 8 tiles per group
NCHUNK = N // 512    # 8 n-chunks of 512 per batch

_cached_nc = None


def _build_nc() -> bass.Bass:
    nc = bacc.Bacc(None, target_bir_lowering=False, debug=False)
    x = nc.dram_tensor("x", [BPC, N, C], F32, kind="ExternalInput")
    # wm / w2m are matmul operands -> declared float32r end-to-end so the
    # BIR verifier sees f32r-rounded producers. wm has an extra zero column
    # so logits row 64 is 0 -> exp gives the ones row that adds w2m's
    # shift row during mm2 (contraction over 65 partitions).
    wm = nc.dram_tensor("wm", [C, K + 1], F32R, kind="ExternalInput")
    w2m = nc.dram_tensor("w2m", [K + 1, C], F32R, kind="ExternalInput")
    y = nc.dram_tensor("y", [BPC, N, C], F32, kind="ExternalOutput")

    with tile.TileContext(nc) as tc, ExitStack() as ctx:
        const = ctx.enter_context(tc.tile_pool(name="const", bufs=1))
        a_pool = ctx.enter_context(tc.tile_pool(name="a", bufs=6))
        at_pool = ctx.enter_context(tc.tile_pool(name="at", bufs=2))
        lg_pool = ctx.enter_context(tc.tile_pool(name="lg", bufs=2))
        attn_pool = ctx.enter_context(tc.tile_pool(name="attn", bufs=2))
        small = ctx.enter_context(tc.tile_pool(name="small", bufs=8))
        tr_psum = ctx.enter_context(tc.tile_pool(name="trps", bufs=3, space="PSUM"))
        l_psum = ctx.enter_context(tc.tile_pool(name="lps", bufs=2, space="PSUM"))
        y_psum = ctx.enter_context(tc.tile_pool(name="yps", bufs=3, space="PSUM"))

        ident = const.tile([128, 128], F32)
        make_identity(nc, ident)
        wm_sb = const.tile([128, 4, K + 1], F32R)  # [p, c4, k] = wm[c4*128+p, k]
        nc.sync.dma_start(out=wm_sb, in_=wm.rearrange("(c4 p) k -> p c4 k", p=128))
        w2m_sb = const.tile([K + 1, C], F32R)
        nc.sync.dma_start(out=w2m_sb, in_=w2m[:, :])

        for b in range(BPC):
            xb = x[b].rearrange("(g t p) c -> g p t c", p=128, t=TPG)
            yb = y[b].rearrange("(g t p) c -> g p t c", p=128, t=TPG)

            a_big = []
            for g in range(NG):
                ag = a_pool.tile([128, TPG, C], F32, tag="a")
                nc.sync.dma_start(out=ag, in_=xb[g])
                a_big.append(ag)

            # ---- logitsT [64, N] = (A @ wm)^T via PE transpose + mm1 ----
            logitsT = lg_pool.tile([K + 1, N], F32, tag="lg")
            for q in range(NCHUNK):  # 512-token chunks
                g, j = divmod(q, 2)
                at_tile = at_pool.tile([128, 4, 512], F32R, tag="at")
                for c4 in range(4):
                    p_tr = tr_psum.tile([128, 512], F32, tag="tr")
                    for tt in range(4):
                        t = 4 * j + tt
                        nc.tensor.transpose(
                            p_tr[:, ts(tt, 128)],
                            a_big[g][:, t, ts(c4, 128)],
                            ident,
                        )
                    # copy A^T chunk out of PSUM (rounds fp32 -> f32r)
                    nc.scalar.copy(at_tile[:, c4], p_tr)
                p_l = l_psum.tile([K + 1, 512], F32, tag="l")
                for c4 in range(4):
                    nc.tensor.matmul(
                        p_l,
                        lhsT=wm_sb[:, c4],
                        rhs=at_tile[:, c4],
                        start=(c4 == 0),
                        stop=(c4 == 3),
                    )
                nc.scalar.copy(logitsT[:, ts(q, 512)], p_l)

            # ---- column softmax over the free (token) axis ----
            negmax = small.tile([K + 1, 1], F32, tag="negmax")
            nc.vector.tensor_reduce(
                out=negmax, in_=logitsT, axis=mybir.AxisListType.X,
                op=mybir.AluOpType.max, negate=True,
            )
            attn = attn_pool.tile([K + 1, N], F32R, tag="attn")
            sums = small.tile([K + 1, 1], F32, tag="sums")
            nc.scalar.activation(
                out=attn, in_=logitsT,
                func=mybir.ActivationFunctionType.Exp,
                bias=negmax, scale=1.0, accum_out=sums,
            )
            rsum = small.tile([K + 1, 1], F32, tag="rsum")
            nc.vector.reciprocal(out=rsum, in_=sums)
            nc.vector.tensor_scalar_mul(attn[0:64], attn[0:64], rsum[0:64])

            # ---- mm2 + residual + relu, written back over A tiles ----
            for g in range(NG):
                for t in range(TPG):
                    nt = g * TPG + t
                    p_y = y_psum.tile([128, C], F32, tag="y")
                    nc.tensor.matmul(
                        p_y,
                        lhsT=attn[:, ts(nt, 128)],
                        rhs=w2m_sb,
                        start=True, stop=True,
                    )
                    nc.vector.tensor_add(p_y, p_y, a_big[g][:, t])
                    nc.scalar.activation(
                        out=a_big[g][:, t], in_=p_y,
                        func=mybir.ActivationFunctionType.Relu,
                    )
                nc.scalar.dma_start(out=yb[g], in_=a_big[g])

    nc.finalize()
    return nc


def _get_nc() -> bass.Bass:
    global _cached_nc
    if _cached_nc is None:
        _cached_nc = _build_nc()
    return _cached_nc


def _fold_weights(w1, m0, m1, w2, gamma, beta, bn_mean, bn_var):
    w1 = np.asarray(w1, np.float64)
    m0 = np.asarray(m0, np.float64)
    m1 = np.asarray(m1, np.float64)
    w2 = np.asarray(w2, np.float64)
    gamma = np.asarray(gamma, np.float64)
    beta = np.asarray(beta, np.float64)
    bn_mean = np.asarray(bn_mean, np.float64)
    bn_var = np.asarray(bn_var, np.float64)

    wm_aug = np.zeros((C, K + 1), np.float32)
    wm_aug[:, :K] = (w1 @ m0).astype(np.float32)  # col K stays 0 -> ones row
    scale = gamma / np.sqrt(bn_var + BN_EPS)
    w2m_aug = np.zeros((K + 1, C), np.float32)
    w2m_aug[:K] = (m1 @ (w2 * scale[None, :])).astype(np.float32)
    w2m_aug[K] = (beta - bn_mean * scale).astype(np.float32)  # shift row
    return wm_aug, w2m_aug


def _run(inputs_np: dict, trace: bool = False):
    nc = _get_nc()
    inp = np.ascontiguousarray(np.asarray(inputs_np["inputs"], np.float32))
    wm, w2m_aug = _fold_weights(
        inputs_np["w1"], inputs_np["m0"], inputs_np["m1"], inputs_np["w2"],
        inputs_np["gamma"], inputs_np["beta"],
        inputs_np["bn_mean"], inputs_np["bn_var"],
    )
    flat = inp.reshape(B, N, C)
    in_maps = [
        {
            "x": np.ascontiguousarray(flat[i * BPC:(i + 1) * BPC]),
            "wm": wm,
            "w2m": w2m_aug,
        }
        for i in range(NCORES)
    ]
    res = run_bass_kernel_spmd(nc, in_maps, core_ids=list(range(NCORES)), trace=trace)
    out = np.concatenate([r["y"] for r in res.results], axis=0)
    return out.reshape(B, H, W, C), res


def kernel(**inputs) -> np.ndarray:
    out, _ = _run(inputs, trace=False)
    return out


# revision 14
# speedup vs baseline: 1.3298x; 1.3298x over previous
"""ExternalAttention Trainium2 Bass kernel.

Math (per batch b, with N = H*W = 4096 tokens, C = 512, K = 64):
    x      = inputs @ w1 + b1          [N, C]
    logits = x @ m0                    [N, K]
    attn   = softmax(logits, axis=N)   (the following L1-normalize over N is a
                                        divide by 1 + 1e-9 -> skipped; the max
                                        subtraction is shift-invariant and
                                        logits are O(1) -> skipped)
    y      = attn @ m1 @ w2            [N, C]
    out    = relu(BN_affine(y) + inputs)

Host-side folds (all tiny C x C / C x K matrices):
    wm    = [w1 @ m0 | 0]                           [C, K+1]  (b1 @ m0 shifts each
            softmax column by a constant -> softmax-invariant, dropped; the zero
            column makes exp produce a ones-row that injects the BN shift)
    scale = gamma / sqrt(bn_var + eps); shift = beta - bn_mean * scale
    w2m   = [m1 @ (w2 * scale) ; shift]             [K+1, C]
    => out = relu(colsoftmax(inputs @ wm_aug) @ w2m_aug + inputs)

Device kernel (per core, 2 batches, data-parallel over B=16 on 8 cores).
Everything is float32r (fp32 bits, PE full-rate). Tokens are interleaved
n = base + p*4 + e so each DMA descriptor moves 8KB contiguous per partition.
    - load A tiles [128, 2, 4, 512] (2MB DMAs on the sync ring)
    - PE-transpose A into A^T psum chunks, copy to SBUF (ACT/DVE),
      mm1 -> logitsT chunk [65, 512] in psum
    - ACT exp straight from psum into attn [65, N] with accumulated row sums;
      DVE reciprocal + per-row scale (column softmax, shift skipped)
    - mm2 per 128-token tile: psum = identity @ A_tile (residual) +
      attn_aug^T @ w2m_aug (shift via ones row), relu (ACT/DVE) back over A
    - store (2MB DMAs on the gpsimd/SWDGE ring)
"""

import os
import sys
from contextlib import ExitStack

import numpy as np

for _p in ("/opt/trn_rl_repo", os.path.expanduser("~/.axon_site/_ro/trn_rl_repo")):
    if os.path.isdir(_p) and _p not in sys.path:
        sys.path.insert(0, _p)

import concourse.bass as bass
import concourse.mybir as mybir
import concourse.tile as tile
from concourse import bacc
from concourse.bass import ts
from concourse.bass_utils import run_bass_kernel_spmd

B, H, W, C, K = 16, 64, 64, 512, 64
N = H * W  # 4096 tokens
BN_EPS = 1e-3
NCORES = 8
BPC = B // NCORES  # batches per core = 2

F32 = mybir.dt.float32
F32R = mybir.dt.float32r

NG = 4               # token groups of 1024 per batch
E = 4                # tokens interleaved per partition (8KB DMA runs)
NCHUNK = N // 512    # 8 n-chunks of 512 per batch; chunk q = (g, t)

_cached_nc = None


def _build_nc() -> bass.Bass:
    nc = bacc.Bacc(None, target_bir_lowering=False, debug=False)
    x = nc.dram_tensor("x", [BPC, N, C], F32R, kind="ExternalInput")
    wm = nc.dram_tensor("wm", [C, K + 1], F32R, kind="ExternalInput")
    w2m = nc.dram_tensor("w2m", [K + 1, C], F32R, kind="ExternalInput")
    ident = nc.dram_tensor("ident", [128, 128], F32R, kind="ExternalInput")
    y = nc.dram_tensor("y", [BPC, N, C], F32R, kind="ExternalOutput")

    with tile.TileContext(nc) as tc, ExitStack() as ctx:
        const = ctx.enter_context(tc.tile_pool(name="const", bufs=1))
        a_pool = ctx.enter_context(tc.tile_pool(name="a", bufs=2 * NG))
        at_pool = ctx.enter_context(tc.tile_pool(name="at", bufs=2))
        attn_pool = ctx.enter_context(tc.tile_pool(name="attn", bufs=2))
        small = ctx.enter_context(tc.tile_pool(name="small", bufs=4))
        tr_psum = ctx.enter_context(tc.tile_pool(name="trps", bufs=2, space="PSUM"))
        l_psum = ctx.enter_context(tc.tile_pool(name="lps", bufs=2, space="PSUM"))
        y_psum = ctx.enter_context(tc.tile_pool(name="yps", bufs=2, space="PSUM"))

        ident_sb = const.tile([128, 128], F32R)
        nc.sync.dma_start(out=ident_sb, in_=ident[:, :])
        wm_sb = const.tile([128, 4, K + 1], F32R)  # [p, c4, k] = wm[c4*128+p, k]
        nc.sync.dma_start(out=wm_sb, in_=wm.rearrange("(c4 p) k -> p c4 k", p=128))
        w2m_sb = const.tile([K + 1, C], F32R)
        nc.sync.dma_start(out=w2m_sb, in_=w2m[:, :])

        copy_flip = [0]

        def pcopy(out_ap, in_ap):
            """PSUM->SBUF copy, alternating ACT/DVE to balance load."""
            copy_flip[0] ^= 1
            if copy_flip[0]:
                nc.scalar.copy(out_ap, in_ap)
            else:
                nc.vector.tensor_copy(out_ap, in_ap)

        for b in range(BPC):
            # token n = g*1024 + t*512 + p*4 + e
            xb = x[b].rearrange("(g t p e) c -> g p t e c", g=NG, t=2, p=128)
            yb = y[b].rearrange("(g t p e) c -> g p t e c", g=NG, t=2, p=128)

            a_big = []
            for g in range(NG):
                ag = a_pool.tile([128, 2, E, C], F32R, tag="a")
                nc.sync.dma_start(out=ag, in_=xb[g])
                a_big.append(ag)

            attn = attn_pool.tile([K + 1, N], F32R, tag="attn")
            sums = small.tile([K + 1, NCHUNK], F32, tag="sums")

            # ---- A^T chunks + mm1 + exp, one 512-token chunk at a time ----
            for q in range(NCHUNK):
                g, t = divmod(q, 2)
                at_tile = at_pool.tile([128, 4, 512], F32R, tag="at")
                for c2 in range(2):  # pairs of 128-channel chunks
                    p_tr = tr_psum.tile([128, 2, 512], F32R, tag="tr")
                    for ci in range(2):
                        c4 = 2 * c2 + ci
                        for e in range(E):
                            nc.tensor.transpose(
                                p_tr[:, ci, ts(e, 128)],
                                a_big[g][:, t, e, ts(c4, 128)],
                                ident_sb,
                            )
                    pcopy(at_tile[:, 2 * c2:2 * c2 + 2], p_tr)
                p_l = l_psum.tile([K + 1, 512], F32, tag="l")
                for c4 in range(4):
                    nc.tensor.matmul(
                        p_l,
                        lhsT=wm_sb[:, c4],
                        rhs=at_tile[:, c4],
                        start=(c4 == 0),
                        stop=(c4 == 3),
                    )
                # exp straight from psum; row K is exp(0)=1 (ones row);
                # per-chunk row sums accumulate into sums[:, q]
                nc.scalar.activation(
                    out=attn[:, ts(q, 512)], in_=p_l,
                    func=mybir.ActivationFunctionType.Exp,
                    accum_out=sums[:, q:q + 1],
                )

            # ---- finish column softmax: scale rows by 1/sum ----
            total = small.tile([K + 1, 1], F32, tag="total")
            nc.vector.reduce_sum(out=total, in_=sums, axis=mybir.AxisListType.X)
            rsum = small.tile([K + 1, 1], F32, tag="rsum")
            nc.vector.reciprocal(out=rsum, in_=total)
            nc.vector.tensor_scalar_mul(attn[0:K], attn[0:K], rsum[0:K])

            # ---- mm2 + residual + relu, written back over A tiles ----
            for g in range(NG):
                for t in range(2):
                    for sub in range(E):
                        nt = (g * 2 + t) * E + sub
                        p_y = y_psum.tile([128, C], F32, tag="y")
                        nc.tensor.matmul(
                            p_y,
                            lhsT=ident_sb,
                            rhs=a_big[g][:, t, sub],
                            start=True, stop=False,
                        )
                        nc.tensor.matmul(
                            p_y,
                            lhsT=attn[:, ts(nt, 128)],
                            rhs=w2m_sb,
                            start=False, stop=True,
                        )
                        if sub % 2 == 0:
                            nc.scalar.activation(
                                out=a_big[g][:, t, sub], in_=p_y,
                                func=mybir.ActivationFunctionType.Relu,
                            )
                        else:
                            nc.vector.tensor_scalar_max(
                                a_big[g][:, t, sub], p_y, 0.0,
                            )
                nc.gpsimd.dma_start(out=yb[g], in_=a_big[g])

    nc.finalize()
    return nc


def _get_nc() -> bass.Bass:
    global _cached_nc
    if _cached_nc is None:
        _cached_nc = _build_nc()
    return _cached_nc


def _fold_weights(w1, m0, m1, w2, gamma, beta, bn_mean, bn_var):
    w1 = np.asarray(w1, np.float64)
    m0 = np.asarray(m0, np.float64)
    m1 = np.asarray(m1, np.float64)
    w2 = np.asarray(w2, np.float64)
    gamma = np.asarray(gamma, np.float64)
    beta = np.asarray(beta, np.float64)
    bn_mean = np.asarray(bn_mean, np.float64)
    bn_var = np.asarray(bn_var, np.float64)

    wm_aug = np.zeros((C, K + 1), np.float32)
    wm_aug[:, :K] = (w1 @ m0).astype(np.float32)  # col K stays 0 -> ones row
    scale = gamma / np.sqrt(bn_var + BN_EPS)
    w2m_aug = np.zeros((K + 1, C), np.float32)
    w2m_aug[:K] = (m1 @ (w2 * scale[None, :])).astype(np.float32)
    w2m_aug[K] = (beta - bn_mean * scale).astype(np.float32)  # shift row
    return wm_aug, w2m_aug


def _run(inputs_np: dict, trace: bool = False):
    nc = _get_nc()
    inp = np.ascontiguousarray(np.asarray(inputs_np["inputs"], np.float32))
    wm_aug, w2m_aug = _fold_weights(
        inputs_np["w1"], inputs_np["m0"], inputs_np["m1"], inputs_np["w2"],
        inputs_np["gamma"], inputs_np["beta"],
        inputs_np["bn_mean"], inputs_np["bn_var"],
    )
    eye = np.eye(128, dtype=np.float32)
    flat = inp.reshape(B, N, C)
    in_maps = [
        {
            "x": np.ascontiguousarray(flat[i * BPC:(i + 1) * BPC]),
            "wm": wm_aug,
            "w2m": w2m_aug,
            "ident": eye,
        }
        for i in range(NCORES)
    ]
    res = run_bass_kernel_spmd(nc, in_maps, core_ids=list(range(NCORES)), trace=trace)
    out = np.concatenate([r["y"] for r in res.results], axis=0)
    return out.reshape(B, H, W, C), res


def kernel(**inputs) -> np.ndarray:
    out, _ = _run(inputs, trace=False)
    return out
